# revision 21
# baseline (speedup 1.0000x reference)
"""MoE layer (E=8 experts, top-2) on 8 Trainium2 NeuronCores.

Strategy: expert-parallel. The gate (logits -> softmax -> top-k) is computed
on the host in fp32 (tiny: [8192,1024]@[1024,8]). Tokens are dispatched to
their top-2 experts on the host (the "all-to-all"), each of the 8 cores runs
one expert's FFN over its gathered tokens (bf16 matmuls, fp32 accumulate,
fused bias + leaky-relu + gate scaling on device), and the host scatter-adds
the returned per-expert outputs back into the full [T, D] output.

All shapes are hardcoded for B=4, S=2048, D=1024, FF=4096, E=8, K=2.
"""

import numpy as np
import ml_dtypes

import concourse.bass as bass
from concourse import bacc
import concourse.mybir as mybir
import concourse.tile as tile
from concourse.bass_utils import run_bass_kernel_spmd

AF = mybir.ActivationFunctionType
BF16 = mybir.dt.bfloat16
F32 = mybir.dt.float32

D = 1024
FF = 4096
E = 8
K = 2
P = 128
ND = D // P    # 8 d-tiles
NF = FF // P   # 32 ff-tiles
BLK = 384      # tokens per matmul free-dim block

_NC_CACHE = {}
LAST_RESULT = None


def _blocks(C):
    """Split capacity C (multiple of 128) into matmul free-dim blocks.
    Block 0 is 384 wide (keeps w1-chunk consumption slower than DMA delivery
    during startup); the rest are 512 (PSUM bank limit) to minimize matmul
    issue count, with the remainder last (small tail epilogue)."""
    blocks = []
    rem = C
    while rem > BLK:
        blocks.append(BLK)
        rem -= BLK
    if rem:
        blocks.append(rem)
    return blocks


# w1 SBUF chunk widths (columns of [P, ND, FF]); tiny leading chunks so the
# first matmul group only waits for 0.25MB of weight DMA.
_W1_CHUNKS = [128, 128, 256] + [512] * 7


def _build_nc(C):
    """Per-core FFN kernel: yT = gates * (prelu(x @ w1.T + b1, 0.1) @ w2.T + b2),
    all tensors in transposed (feature-on-partition) layout."""
    blocks = _blocks(C)
    nc = bacc.Bacc("TRN2", target_bir_lowering=False, debug=False, num_devices=8)

    xT = nc.dram_tensor("xT", [P, ND, C], BF16, kind="ExternalInput")
    w1t = nc.dram_tensor("w1t", [P, ND, FF], BF16, kind="ExternalInput")
    w2t = nc.dram_tensor("w2t", [P, NF, D], BF16, kind="ExternalInput")
    b1d = nc.dram_tensor("b1d", [P, NF], F32, kind="ExternalInput")
    b2d = nc.dram_tensor("b2d", [P, ND], F32, kind="ExternalInput")
    gd = nc.dram_tensor("gd", [P, C], F32, kind="ExternalInput")
    yT = nc.dram_tensor("yT", [P, ND, C], F32, kind="ExternalOutput")

    with tile.TileContext(nc) as tc:
        with (
            tc.tile_pool(name="weights", bufs=1) as wpool,
            tc.tile_pool(name="xin", bufs=2) as xpool,
            tc.tile_pool(name="hmid", bufs=1) as hpool,
            tc.tile_pool(name="yout", bufs=2) as ypool,
            tc.tile_pool(name="psum", bufs=8, space="PSUM") as psum,
        ):
            # DMA emission order matters: all inputs share one FIFO queue
            # (drained by 16 engines at ~330GB/s), so order by first use: x
            # block 0 and the leading w1 chunks first, w2/gates deferred.
            blk0 = blocks[0]
            x0_sb = xpool.tile([P, ND, blk0], BF16, tag="x")
            nc.sync.dma_start(x0_sb[:], xT[:, :, 0:blk0])
            # (chunk index, col offset within chunk) for each ff tile
            w1_map = []
            w1_sb = []
            col = 0
            for i, w in enumerate(_W1_CHUNKS):
                t = wpool.tile([P, ND, w], BF16, tag=f"w1_{i}")
                nc.sync.dma_start(t[:], w1t[:, :, col:col + w])
                w1_sb.append(t)
                for fc in range(0, w, P):
                    w1_map.append((i, fc))
                col += w
                if i == 0:
                    b1_sb = wpool.tile([P, NF], F32, tag="b1")
                    nc.sync.dma_start(b1_sb[:], b1d[:])
            b2_sb = wpool.tile([P, ND], F32, tag="b2")
            nc.sync.dma_start(b2_sb[:], b2d[:])
            g_sb = wpool.tile([P, C], F32, tag="g")
            nc.sync.dma_start(g_sb[:], gd[:])
            w2_sb = []
            for dd in range(ND):
                t = wpool.tile([P, NF, P], BF16, tag=f"w2_{dd}")
                nc.sync.dma_start(t[:], w2t[:, :, dd * P:(dd + 1) * P])
                w2_sb.append(t)

            c0 = 0
            for blk, blkw in enumerate(blocks):
                if blk == 0:
                    x_sb = x0_sb
                else:
                    x_sb = xpool.tile([P, ND, blkw], BF16, tag="x")
                    nc.sync.dma_start(x_sb[:], xT[:, :, c0:c0 + blkw])

                h_sb = hpool.tile([P, NF, blkw], BF16, tag="h")
                # h.T tiles: [FF-part, blkw] = sum_d w1t[d,ff].T @ xT[d]
                for ff in range(NF):
                    ps = psum.tile([P, blkw], F32, tag="ps")
                    ci, fc = w1_map[ff]
                    wch = w1_sb[ci]
                    for dd in range(ND):
                        nc.tensor.matmul(
                            ps[:],
                            wch[:, dd, fc:fc + P],
                            x_sb[:, dd, :],
                            start=(dd == 0),
                            stop=(dd == ND - 1),
                        )
                    # h = prelu(ps + b1, 0.1), cast to bf16
                    nc.scalar.activation(
                        h_sb[:, ff, :], ps[:], AF.Prelu,
                        bias=b1_sb[:, ff:ff + 1], scale=1.0, alpha=0.1,
                    )

                y_sb = ypool.tile([P, ND, blkw], F32, tag="y")
                # y.T tiles: [D-part, blkw] = sum_ff w2t[ff,d].T @ hT[ff]
                for dd in range(ND):
                    ps2 = psum.tile([P, blkw], F32, tag="ps")
                    for ff in range(NF):
                        nc.tensor.matmul(
                            ps2[:],
                            w2_sb[dd][:, ff, :],
                            h_sb[:, ff, :],
                            start=(ff == 0),
                            stop=(ff == NF - 1),
                        )
                    # y = (ps2 + b2) * gates
                    nc.scalar.activation(
                        y_sb[:, dd, :], ps2[:], AF.Identity,
                        bias=b2_sb[:, dd:dd + 1], scale=1.0,
                    )
                    nc.vector.tensor_mul(
                        y_sb[:, dd, :], y_sb[:, dd, :], g_sb[:, c0:c0 + blkw]
                    )
                    # per-d out DMA: stream results out while later d-groups
                    # are still in the matmul pipeline
                    nc.sync.dma_start(yT[:, dd, c0:c0 + blkw], y_sb[:, dd, :])
                c0 += blkw
    nc.compile()
    return nc


def _route(x_flat, w_gate, b_gate):
    """Host gate: fp32 logits/softmax/top-2, matching jax.lax.top_k tie-breaking."""
    logits = x_flat @ w_gate.T + b_gate
    m = logits.max(axis=-1, keepdims=True)
    e = np.exp(logits - m, dtype=np.float32)
    scores = e / e.sum(axis=-1, keepdims=True, dtype=np.float32)
    order = np.argsort(-scores, axis=-1, kind="stable")
    topk_idx = order[:, :K].astype(np.int32)
    topk_vals = np.take_along_axis(scores, topk_idx, axis=-1).astype(np.float32)
    return topk_idx, topk_vals


def kernel(x, w_gate, b_gate, w1, b1, w2, b2):
    B, S, _ = x.shape
    T = B * S
    x_flat = np.ascontiguousarray(x, dtype=np.float32).reshape(T, D)

    topk_idx, topk_vals = _route(x_flat, np.asarray(w_gate, np.float32),
                                 np.asarray(b_gate, np.float32))

    # Dispatch: token ids + gate weight per expert.
    ids_list, gv_list = [], []
    for e in range(E):
        sel = (topk_idx == e)
        mask = sel.any(axis=1)
        ids = np.nonzero(mask)[0]
        gv = topk_vals[ids][sel[ids]]  # gate value of the slot matching expert e
        ids_list.append(ids)
        gv_list.append(gv.astype(np.float32))

    max_count = max(len(i) for i in ids_list)
    C = max(-(-max_count // P) * P, 256)  # round up to 128 multiple

    if C not in _NC_CACHE:
        _NC_CACHE[C] = _build_nc(C)
    nc = _NC_CACHE[C]

    in_maps = []
    for e in range(E):
        ids = ids_list[e]
        n = len(ids)
        xg = np.zeros((C, D), dtype=np.float32)
        xg[:n] = x_flat[ids]
        # [C, D] -> [128, ND, C] with [p, d, c] = xg[c, d*128+p]
        xT = np.ascontiguousarray(
            xg.T.reshape(ND, P, C).transpose(1, 0, 2)).astype(ml_dtypes.bfloat16)
        w1t = np.ascontiguousarray(
            np.asarray(w1[e], np.float32).T.reshape(ND, P, FF).transpose(1, 0, 2)
        ).astype(ml_dtypes.bfloat16)
        w2t = np.ascontiguousarray(
            np.asarray(w2[e], np.float32).T.reshape(NF, P, D).transpose(1, 0, 2)
        ).astype(ml_dtypes.bfloat16)
        b1d = np.ascontiguousarray(np.asarray(b1[e], np.float32).reshape(NF, P).T)
        b2d = np.ascontiguousarray(np.asarray(b2[e], np.float32).reshape(ND, P).T)
        g = np.zeros((C,), dtype=np.float32)
        g[:n] = gv_list[e]
        gd = np.ascontiguousarray(np.broadcast_to(g, (P, C)))
        in_maps.append({"xT": xT, "w1t": w1t, "w2t": w2t,
                        "b1d": b1d, "b2d": b2d, "gd": gd})

    res = run_bass_kernel_spmd(nc, in_maps, core_ids=list(range(8)))
    global LAST_RESULT
    LAST_RESULT = res

    out_flat = np.zeros((T, D), dtype=np.float32)
    for e in range(E):
        ids = ids_list[e]
        n = len(ids)
        yT = res.results[e]["yT"]  # [128, ND, C] f32
        y = yT.transpose(1, 0, 2).reshape(D, C).T  # [C, D]
        out_flat[ids] += y[:n]

    return out_flat.reshape(B, S, D), topk_idx, topk_vals


# revision 22
# speedup vs baseline: 1.0080x; 1.0080x over previous
"""MoE layer (E=8 experts, top-2) on 8 Trainium2 NeuronCores.

Strategy: expert-parallel. The gate (logits -> softmax -> top-k) is computed
on the host in fp32 (tiny: [8192,1024]@[1024,8]). Tokens are dispatched to
their top-2 experts on the host (the "all-to-all"), each of the 8 cores runs
one expert's FFN over its gathered tokens (bf16 matmuls, fp32 accumulate,
fused bias + leaky-relu + gate scaling on device), and the host scatter-adds
the returned per-expert outputs back into the full [T, D] output.

All shapes are hardcoded for B=4, S=2048, D=1024, FF=4096, E=8, K=2.
"""

import numpy as np
import ml_dtypes

import concourse.bass as bass
from concourse import bacc
import concourse.mybir as mybir
import concourse.tile as tile
from concourse.bass_utils import run_bass_kernel_spmd

AF = mybir.ActivationFunctionType
BF16 = mybir.dt.bfloat16
F32 = mybir.dt.float32

D = 1024
FF = 4096
E = 8
K = 2
P = 128
ND = D // P    # 8 d-tiles
NF = FF // P   # 32 ff-tiles
BLK = 384      # tokens per matmul free-dim block

_NC_CACHE = {}
LAST_RESULT = None


def _blocks(C):
    """Split capacity C (multiple of 128) into matmul free-dim blocks.
    Block 0 is 384 wide (keeps w1-chunk consumption slower than DMA delivery
    during startup); the rest are 512 (PSUM bank limit) to minimize matmul
    issue count, with the remainder last (small tail epilogue)."""
    blocks = []
    rem = C
    while rem > BLK:
        blocks.append(BLK)
        rem -= BLK
    if rem:
        blocks.append(rem)
    return blocks


# w1 SBUF chunk widths (columns of [P, ND, FF]); tiny leading chunks so the
# first matmul group only waits for 0.25MB of weight DMA.
_W1_CHUNKS = [128, 128, 256] + [512] * 7


def _build_nc(C):
    """Per-core FFN kernel: yT = gates * (prelu(x @ w1.T + b1, 0.1) @ w2.T + b2),
    all tensors in transposed (feature-on-partition) layout."""
    blocks = _blocks(C)
    nc = bacc.Bacc("TRN2", target_bir_lowering=False, debug=False, num_devices=8)

    xT = nc.dram_tensor("xT", [P, ND, C], BF16, kind="ExternalInput")
    w1t = nc.dram_tensor("w1t", [P, ND, FF], BF16, kind="ExternalInput")
    w2t = nc.dram_tensor("w2t", [P, NF, D], BF16, kind="ExternalInput")
    b1d = nc.dram_tensor("b1d", [P, NF], F32, kind="ExternalInput")
    b2d = nc.dram_tensor("b2d", [P, ND], F32, kind="ExternalInput")
    gd = nc.dram_tensor("gd", [P, C], F32, kind="ExternalInput")
    yT = nc.dram_tensor("yT", [P, ND, C], F32, kind="ExternalOutput")

    with tile.TileContext(nc) as tc:
        with (
            tc.tile_pool(name="weights", bufs=1) as wpool,
            tc.tile_pool(name="xin", bufs=2) as xpool,
            tc.tile_pool(name="hmid", bufs=1) as hpool,
            tc.tile_pool(name="yout", bufs=2) as ypool,
            tc.tile_pool(name="psum", bufs=8, space="PSUM") as psum,
        ):
            # DMA emission order matters: all inputs share one FIFO queue
            # (drained by 16 engines at ~330GB/s), so order by first use: x
            # block 0 and the leading w1 chunks first, w2/gates deferred.
            blk0 = blocks[0]
            x0_sb = xpool.tile([P, ND, blk0], BF16, tag="x")
            nc.sync.dma_start(x0_sb[:], xT[:, :, 0:blk0])
            # (chunk index, col offset within chunk) for each ff tile
            w1_map = []
            w1_sb = []
            col = 0
            for i, w in enumerate(_W1_CHUNKS):
                t = wpool.tile([P, ND, w], BF16, tag=f"w1_{i}")
                nc.sync.dma_start(t[:], w1t[:, :, col:col + w])
                w1_sb.append(t)
                for fc in range(0, w, P):
                    w1_map.append((i, fc))
                col += w
                if i == 0:
                    b1_sb = wpool.tile([P, NF], F32, tag="b1")
                    nc.sync.dma_start(b1_sb[:], b1d[:])
            b2_sb = wpool.tile([P, ND], F32, tag="b2")
            nc.sync.dma_start(b2_sb[:], b2d[:])
            g_sb = wpool.tile([P, C], F32, tag="g")
            nc.sync.dma_start(g_sb[:], gd[:])
            w2_sb = []
            for dd in range(ND):
                t = wpool.tile([P, NF, P], BF16, tag=f"w2_{dd}")
                nc.sync.dma_start(t[:], w2t[:, :, dd * P:(dd + 1) * P])
                w2_sb.append(t)

            c0 = 0
            for blk, blkw in enumerate(blocks):
                if blk == 0:
                    x_sb = x0_sb
                else:
                    x_sb = xpool.tile([P, ND, blkw], BF16, tag="x")
                    nc.sync.dma_start(x_sb[:], xT[:, :, c0:c0 + blkw])

                h_sb = hpool.tile([P, NF, blkw], BF16, tag="h")
                # h.T tiles: [FF-part, blkw] = sum_d w1t[d,ff].T @ xT[d]
                for ff in range(NF):
                    ps = psum.tile([P, blkw], F32, tag="ps")
                    ci, fc = w1_map[ff]
                    wch = w1_sb[ci]
                    for dd in range(ND):
                        nc.tensor.matmul(
                            ps[:],
                            wch[:, dd, fc:fc + P],
                            x_sb[:, dd, :],
                            start=(dd == 0),
                            stop=(dd == ND - 1),
                        )
                    # h = prelu(ps + b1, 0.1), cast to bf16
                    nc.scalar.activation(
                        h_sb[:, ff, :], ps[:], AF.Prelu,
                        bias=b1_sb[:, ff:ff + 1], scale=1.0, alpha=0.1,
                    )

                y_sb = ypool.tile([P, ND, blkw], F32, tag="y")
                # y.T tiles: [D-part, blkw] = sum_ff w2t[ff,d].T @ hT[ff]
                for dd in range(ND):
                    ps2 = psum.tile([P, blkw], F32, tag="ps")
                    for ff in range(NF):
                        nc.tensor.matmul(
                            ps2[:],
                            w2_sb[dd][:, ff, :],
                            h_sb[:, ff, :],
                            start=(ff == 0),
                            stop=(ff == NF - 1),
                        )
                    # y = (ps2 + b2) * gates
                    nc.scalar.activation(
                        y_sb[:, dd, :], ps2[:], AF.Identity,
                        bias=b2_sb[:, dd:dd + 1], scale=1.0,
                    )
                    nc.vector.tensor_mul(
                        y_sb[:, dd, :], y_sb[:, dd, :], g_sb[:, c0:c0 + blkw]
                    )
                nc.sync.dma_start(yT[:, :, c0:c0 + blkw], y_sb[:])
                c0 += blkw
    nc.compile()
    return nc


def _route(x_flat, w_gate, b_gate):
    """Host gate: fp32 logits/softmax/top-2, matching jax.lax.top_k tie-breaking."""
    logits = x_flat @ w_gate.T + b_gate
    m = logits.max(axis=-1, keepdims=True)
    e = np.exp(logits - m, dtype=np.float32)
    scores = e / e.sum(axis=-1, keepdims=True, dtype=np.float32)
    order = np.argsort(-scores, axis=-1, kind="stable")
    topk_idx = order[:, :K].astype(np.int32)
    topk_vals = np.take_along_axis(scores, topk_idx, axis=-1).astype(np.float32)
    return topk_idx, topk_vals


def kernel(x, w_gate, b_gate, w1, b1, w2, b2):
    B, S, _ = x.shape
    T = B * S
    x_flat = np.ascontiguousarray(x, dtype=np.float32).reshape(T, D)

    topk_idx, topk_vals = _route(x_flat, np.asarray(w_gate, np.float32),
                                 np.asarray(b_gate, np.float32))

    # Dispatch: token ids + gate weight per expert.
    ids_list, gv_list = [], []
    for e in range(E):
        sel = (topk_idx == e)
        mask = sel.any(axis=1)
        ids = np.nonzero(mask)[0]
        gv = topk_vals[ids][sel[ids]]  # gate value of the slot matching expert e
        ids_list.append(ids)
        gv_list.append(gv.astype(np.float32))

    max_count = max(len(i) for i in ids_list)
    C = max(-(-max_count // P) * P, 256)  # round up to 128 multiple

    if C not in _NC_CACHE:
        _NC_CACHE[C] = _build_nc(C)
    nc = _NC_CACHE[C]

    in_maps = []
    for e in range(E):
        ids = ids_list[e]
        n = len(ids)
        xg = np.zeros((C, D), dtype=np.float32)
        xg[:n] = x_flat[ids]
        # [C, D] -> [128, ND, C] with [p, d, c] = xg[c, d*128+p]
        xT = np.ascontiguousarray(
            xg.T.reshape(ND, P, C).transpose(1, 0, 2)).astype(ml_dtypes.bfloat16)
        w1t = np.ascontiguousarray(
            np.asarray(w1[e], np.float32).T.reshape(ND, P, FF).transpose(1, 0, 2)
        ).astype(ml_dtypes.bfloat16)
        w2t = np.ascontiguousarray(
            np.asarray(w2[e], np.float32).T.reshape(NF, P, D).transpose(1, 0, 2)
        ).astype(ml_dtypes.bfloat16)
        b1d = np.ascontiguousarray(np.asarray(b1[e], np.float32).reshape(NF, P).T)
        b2d = np.ascontiguousarray(np.asarray(b2[e], np.float32).reshape(ND, P).T)
        g = np.zeros((C,), dtype=np.float32)
        g[:n] = gv_list[e]
        gd = np.ascontiguousarray(np.broadcast_to(g, (P, C)))
        in_maps.append({"xT": xT, "w1t": w1t, "w2t": w2t,
                        "b1d": b1d, "b2d": b2d, "gd": gd})

    res = run_bass_kernel_spmd(nc, in_maps, core_ids=list(range(8)))
    global LAST_RESULT
    LAST_RESULT = res

    out_flat = np.zeros((T, D), dtype=np.float32)
    for e in range(E):
        ids = ids_list[e]
        n = len(ids)
        yT = res.results[e]["yT"]  # [128, ND, C] f32
        y = yT.transpose(1, 0, 2).reshape(D, C).T  # [C, D]
        out_flat[ids] += y[:n]

    return out_flat.reshape(B, S, D), topk_idx, topk_vals


# revision 24
# speedup vs baseline: 1.0183x; 1.0102x over previous
"""MoE layer (E=8 experts, top-2) on 8 Trainium2 NeuronCores.

Strategy: expert-parallel. The gate (logits -> softmax -> top-k) is computed
on the host in fp32 (tiny: [8192,1024]@[1024,8]). Tokens are dispatched to
their top-2 experts on the host (the "all-to-all"), each of the 8 cores runs
one expert's FFN over its gathered tokens (bf16 matmuls, fp32 accumulate,
fused bias + leaky-relu + gate scaling on device), and the host scatter-adds
the returned per-expert outputs back into the full [T, D] output.

All shapes are hardcoded for B=4, S=2048, D=1024, FF=4096, E=8, K=2.
"""

import numpy as np
import ml_dtypes

import concourse.bass as bass
from concourse import bacc
import concourse.mybir as mybir
import concourse.tile as tile
from concourse.bass_utils import run_bass_kernel_spmd

AF = mybir.ActivationFunctionType
BF16 = mybir.dt.bfloat16
F32 = mybir.dt.float32

D = 1024
FF = 4096
E = 8
K = 2
P = 128
ND = D // P    # 8 d-tiles
NF = FF // P   # 32 ff-tiles
BLK = 384      # tokens per matmul free-dim block

_NC_CACHE = {}
LAST_RESULT = None


def _blocks(C):
    """Split capacity C (multiple of 128) into matmul free-dim blocks.
    Block 0 is 384 wide (keeps w1-chunk consumption slower than DMA delivery
    during startup); the rest are 512 (PSUM bank limit) to minimize matmul
    issue count, with the remainder last (small tail epilogue)."""
    blocks = []
    rem = C
    while rem > BLK:
        blocks.append(BLK)
        rem -= BLK
    if rem:
        blocks.append(rem)
    return blocks


# w1 SBUF chunk widths (columns of [P, ND, FF]); tiny leading chunks so the
# first matmul group only waits for 0.25MB of weight DMA.
_W1_CHUNKS = [128, 128, 256] + [512] * 7


def _build_nc(C):
    """Per-core FFN kernel: yT = gates * (prelu(x @ w1.T + b1, 0.1) @ w2.T + b2),
    all tensors in transposed (feature-on-partition) layout."""
    blocks = _blocks(C)
    nc = bacc.Bacc("TRN2", target_bir_lowering=False, debug=False, num_devices=8)

    xT = nc.dram_tensor("xT", [P, ND, C], BF16, kind="ExternalInput")
    w1t = nc.dram_tensor("w1t", [P, ND, FF], BF16, kind="ExternalInput")
    w2t = nc.dram_tensor("w2t", [P, NF, D], BF16, kind="ExternalInput")
    b1d = nc.dram_tensor("b1d", [P, NF], F32, kind="ExternalInput")
    b2d = nc.dram_tensor("b2d", [P, ND], F32, kind="ExternalInput")
    gd = nc.dram_tensor("gd", [P, C], F32, kind="ExternalInput")
    yT = nc.dram_tensor("yT", [P, ND, C], F32, kind="ExternalOutput")

    with tile.TileContext(nc) as tc:
        with (
            tc.tile_pool(name="weights", bufs=1) as wpool,
            tc.tile_pool(name="xin", bufs=2) as xpool,
            tc.tile_pool(name="hmid", bufs=1) as hpool,
            tc.tile_pool(name="yout", bufs=2) as ypool,
            tc.tile_pool(name="psum", bufs=8, space="PSUM") as psum,
        ):
            # DMA emission order matters: all inputs share one FIFO queue
            # (drained by 16 engines at ~330GB/s), so order by first use: x
            # block 0 and the leading w1 chunks first, w2/gates deferred.
            blk0 = blocks[0]
            x0_sb = xpool.tile([P, ND, blk0], BF16, tag="x")
            nc.sync.dma_start(x0_sb[:], xT[:, :, 0:blk0])
            # (chunk index, col offset within chunk) for each ff tile
            w1_map = []
            w1_sb = []
            col = 0
            for i, w in enumerate(_W1_CHUNKS):
                t = wpool.tile([P, ND, w], BF16, tag=f"w1_{i}")
                nc.sync.dma_start(t[:], w1t[:, :, col:col + w])
                w1_sb.append(t)
                for fc in range(0, w, P):
                    w1_map.append((i, fc))
                col += w
                if i == 0:
                    b1_sb = wpool.tile([P, NF], F32, tag="b1")
                    nc.sync.dma_start(b1_sb[:], b1d[:])
            b2_sb = wpool.tile([P, ND], F32, tag="b2")
            nc.sync.dma_start(b2_sb[:], b2d[:])
            g_sb = wpool.tile([P, C], F32, tag="g")
            nc.sync.dma_start(g_sb[:], gd[:])
            w2_sb = []
            for dd in range(ND):
                t = wpool.tile([P, NF, P], BF16, tag=f"w2_{dd}")
                nc.sync.dma_start(t[:], w2t[:, :, dd * P:(dd + 1) * P])
                w2_sb.append(t)

            c0 = 0
            for blk, blkw in enumerate(blocks):
                if blk == 0:
                    x_sb = x0_sb
                else:
                    x_sb = xpool.tile([P, ND, blkw], BF16, tag="x")
                    nc.sync.dma_start(x_sb[:], xT[:, :, c0:c0 + blkw])

                h_sb = hpool.tile([P, NF, blkw], BF16, tag="h")
                # h.T tiles: [FF-part, blkw] = sum_d w1t[d,ff].T @ xT[d]
                for ff in range(NF):
                    ps = psum.tile([P, blkw], F32, tag="ps")
                    ci, fc = w1_map[ff]
                    wch = w1_sb[ci]
                    for dd in range(ND):
                        nc.tensor.matmul(
                            ps[:],
                            wch[:, dd, fc:fc + P],
                            x_sb[:, dd, :],
                            start=(dd == 0),
                            stop=(dd == ND - 1),
                        )
                    # h = prelu(ps + b1, 0.1), cast to bf16
                    nc.scalar.activation(
                        h_sb[:, ff, :], ps[:], AF.Prelu,
                        bias=b1_sb[:, ff:ff + 1], scale=1.0, alpha=0.1,
                    )

                y_sb = ypool.tile([P, ND, blkw], F32, tag="y")
                # y.T tiles: [D-part, blkw] = sum_ff w2t[ff,d].T @ hT[ff]
                for dd in range(ND):
                    ps2 = psum.tile([P, blkw], F32, tag="ps")
                    for ff in range(NF):
                        nc.tensor.matmul(
                            ps2[:],
                            w2_sb[dd][:, ff, :],
                            h_sb[:, ff, :],
                            start=(ff == 0),
                            stop=(ff == NF - 1),
                        )
                    # y = (ps2 + b2) * gates
                    nc.scalar.activation(
                        y_sb[:, dd, :], ps2[:], AF.Identity,
                        bias=b2_sb[:, dd:dd + 1], scale=1.0,
                    )
                    nc.vector.tensor_mul(
                        y_sb[:, dd, :], y_sb[:, dd, :], g_sb[:, c0:c0 + blkw]
                    )
                    if blk == len(blocks) - 1:
                        # last block: per-d out DMA so the transfer overlaps
                        # the remaining d-groups instead of extending the tail
                        nc.sync.dma_start(yT[:, dd, c0:c0 + blkw], y_sb[:, dd, :])
                if blk != len(blocks) - 1:
                    nc.sync.dma_start(yT[:, :, c0:c0 + blkw], y_sb[:])
                c0 += blkw
    nc.compile()
    return nc


def _route(x_flat, w_gate, b_gate):
    """Host gate: fp32 logits/softmax/top-2, matching jax.lax.top_k tie-breaking."""
    logits = x_flat @ w_gate.T + b_gate
    m = logits.max(axis=-1, keepdims=True)
    e = np.exp(logits - m, dtype=np.float32)
    scores = e / e.sum(axis=-1, keepdims=True, dtype=np.float32)
    order = np.argsort(-scores, axis=-1, kind="stable")
    topk_idx = order[:, :K].astype(np.int32)
    topk_vals = np.take_along_axis(scores, topk_idx, axis=-1).astype(np.float32)
    return topk_idx, topk_vals


def kernel(x, w_gate, b_gate, w1, b1, w2, b2):
    B, S, _ = x.shape
    T = B * S
    x_flat = np.ascontiguousarray(x, dtype=np.float32).reshape(T, D)

    topk_idx, topk_vals = _route(x_flat, np.asarray(w_gate, np.float32),
                                 np.asarray(b_gate, np.float32))

    # Dispatch: token ids + gate weight per expert.
    ids_list, gv_list = [], []
    for e in range(E):
        sel = (topk_idx == e)
        mask = sel.any(axis=1)
        ids = np.nonzero(mask)[0]
        gv = topk_vals[ids][sel[ids]]  # gate value of the slot matching expert e
        ids_list.append(ids)
        gv_list.append(gv.astype(np.float32))

    # matmul free-dim needs no alignment: capacity = exact max expert load
    max_count = max(len(i) for i in ids_list)
    C = max(max_count, 256)

    if C not in _NC_CACHE:
        _NC_CACHE[C] = _build_nc(C)
    nc = _NC_CACHE[C]

    in_maps = []
    for e in range(E):
        ids = ids_list[e]
        n = len(ids)
        xg = np.zeros((C, D), dtype=np.float32)
        xg[:n] = x_flat[ids]
        # [C, D] -> [128, ND, C] with [p, d, c] = xg[c, d*128+p]
        xT = np.ascontiguousarray(
            xg.T.reshape(ND, P, C).transpose(1, 0, 2)).astype(ml_dtypes.bfloat16)
        w1t = np.ascontiguousarray(
            np.asarray(w1[e], np.float32).T.reshape(ND, P, FF).transpose(1, 0, 2)
        ).astype(ml_dtypes.bfloat16)
        w2t = np.ascontiguousarray(
            np.asarray(w2[e], np.float32).T.reshape(NF, P, D).transpose(1, 0, 2)
        ).astype(ml_dtypes.bfloat16)
        b1d = np.ascontiguousarray(np.asarray(b1[e], np.float32).reshape(NF, P).T)
        b2d = np.ascontiguousarray(np.asarray(b2[e], np.float32).reshape(ND, P).T)
        g = np.zeros((C,), dtype=np.float32)
        g[:n] = gv_list[e]
        gd = np.ascontiguousarray(np.broadcast_to(g, (P, C)))
        in_maps.append({"xT": xT, "w1t": w1t, "w2t": w2t,
                        "b1d": b1d, "b2d": b2d, "gd": gd})

    res = run_bass_kernel_spmd(nc, in_maps, core_ids=list(range(8)))
    global LAST_RESULT
    LAST_RESULT = res

    out_flat = np.zeros((T, D), dtype=np.float32)
    for e in range(E):
        ids = ids_list[e]
        n = len(ids)
        yT = res.results[e]["yT"]  # [128, ND, C] f32
        y = yT.transpose(1, 0, 2).reshape(D, C).T  # [C, D]
        out_flat[ids] += y[:n]

    return out_flat.reshape(B, S, D), topk_idx, topk_vals
